# revision 13
# baseline (speedup 1.0000x reference)
# v8: v6 + full DMA buffering (no compute-gated DMA issues), single
# accumulated chunk-sum buffer (one strided sub per piece, mul+reduce once
# at the end), and the last row-tile split into chunk-aligned halves so only
# a half-scan trails the final byte.

import sys

if "/opt/trn_rl_repo" not in sys.path:
    sys.path.insert(0, "/opt/trn_rl_repo")

import numpy as np

import concourse.bass as bass
from concourse import mybir
from concourse.bass_utils import run_bass_kernel_spmd

N_CORES = 8
N_ROWS = 8192
K = 2500
CHUNK = 100
P = 128
ROWS_PER_CORE = N_ROWS // N_CORES   # 1024
ROWS_PER_TILE = P                   # 128
N_TILES = ROWS_PER_CORE // ROWS_PER_TILE  # 8
N_CHUNKS_TOTAL = N_TILES * (K // CHUNK)   # 200
SCALE = 1.0 / (CHUNK * np.sqrt(K // CHUNK))
SCALE2 = float(SCALE * SCALE)

# Pieces: (row_tile, col_start, width). Last tile split into chunk-aligned
# halves so the post-stream tail is a half-scan.
PIECES = [(t, 0, K) for t in range(N_TILES - 1)]
PIECES += [(N_TILES - 1, 0, 1200), (N_TILES - 1, 1200, 1300)]
N_PIECES = len(PIECES)  # 9

_CACHED_NC = None


def _build_nc():
    nc = bass.Bass(
        "TRN2",
        target_bir_lowering=False,
        debug=False,
        num_devices=N_CORES,
    )
    x = nc.dram_tensor(
        "input", [ROWS_PER_CORE, K], mybir.dt.float32, kind="ExternalInput"
    ).ap()
    s = nc.dram_tensor(
        "style", [ROWS_PER_CORE, K], mybir.dt.float32, kind="ExternalInput"
    ).ap()
    o = nc.dram_tensor(
        "out", [P, 1], mybir.dt.float32, kind="ExternalOutput"
    ).ap()

    def src(t_ap, p):
        t, c0, w = PIECES[p]
        r0 = t * ROWS_PER_TILE
        return t_ap[r0 : r0 + ROWS_PER_TILE, c0 : c0 + w]

    from contextlib import ExitStack

    with ExitStack() as ctx:
        # One dedicated buffer per piece: every DMA issues immediately and
        # the rings stay fully fed (slot-reuse waits starved them in v7).
        xt_f = ctx.enter_context(
            nc.sbuf_tensor("xt_f", [P, N_TILES - 1, K], mybir.dt.float32)
        )
        st_f = ctx.enter_context(
            nc.sbuf_tensor("st_f", [P, N_TILES - 1, K], mybir.dt.float32)
        )
        xt_a = ctx.enter_context(nc.sbuf_tensor("xt_a", [P, 1200], mybir.dt.float32))
        st_a = ctx.enter_context(nc.sbuf_tensor("st_a", [P, 1200], mybir.dt.float32))
        xt_b = ctx.enter_context(nc.sbuf_tensor("xt_b", [P, 1300], mybir.dt.float32))
        st_b = ctx.enter_context(nc.sbuf_tensor("st_b", [P, 1300], mybir.dt.float32))
        # sc col 0 is a permanent zero so chunk sums are one strided sub.
        sc = ctx.enter_context(nc.sbuf_tensor("sc", [P, K + 1], mybir.dt.float32))
        csall = ctx.enter_context(
            nc.sbuf_tensor("csall", [P, N_CHUNKS_TOTAL], mybir.dt.float32)
        )
        sqall = ctx.enter_context(
            nc.sbuf_tensor("sqall", [P, N_CHUNKS_TOTAL], mybir.dt.float32)
        )
        partials = ctx.enter_context(
            nc.sbuf_tensor("partials", [P, 1], mybir.dt.float32)
        )
        s_in = [
            ctx.enter_context(nc.semaphore(f"s_in{p}")) for p in range(N_PIECES)
        ]
        s_st = [
            ctx.enter_context(nc.semaphore(f"s_st{p}")) for p in range(N_PIECES)
        ]
        s_done = ctx.enter_context(nc.semaphore("s_done"))
        s_out = ctx.enter_context(nc.semaphore("s_out"))
        block = ctx.enter_context(nc.Block(no_gpsimd_drain=True))

        def xdst(p):
            t, c0, w = PIECES[p]
            if p < N_TILES - 1:
                return xt_f[:, p, :]
            return xt_a[:] if w == 1200 else xt_b[:]

        def sdst(p):
            t, c0, w = PIECES[p]
            if p < N_TILES - 1:
                return st_f[:, p, :]
            return st_a[:] if w == 1200 else st_b[:]

        @block.sync
        def _(sync):
            for p in range(N_PIECES):
                sync.dma_start(out=xdst(p), in_=src(x, p)).then_inc(s_in[p], 16)
            sync.wait_ge(s_done, 1)
            sync.dma_start(out=o, in_=partials[:]).then_inc(s_out, 16)
            sync.wait_ge(s_out, 16)

        @block.scalar
        def _(scalar):
            for p in range(N_PIECES):
                scalar.dma_start(out=sdst(p), in_=src(s, p)).then_inc(
                    s_st[p], 16
                )

        @block.vector
        def _(vector):
            nc.vector.memset(sc[:, 0:1], 0.0)
            cs_off = 0
            for p in range(N_PIECES):
                w = PIECES[p][2]
                nchunk = w // CHUNK
                vector.wait_ge(s_in[p], 16)
                vector.wait_ge(s_st[p], 16)
                # sc[:, 1+j] = sum_{i<=j} (x[:, i] - s[:, i])  (fp32 state)
                nc.vector.tensor_tensor_scan(
                    out=sc[:, 1 : w + 1],
                    data0=xdst(p),
                    data1=sdst(p),
                    initial=0.0,
                    op0=mybir.AluOpType.add,
                    op1=mybir.AluOpType.subtract,
                )
                vector.drain()
                # chunk sums: cs[c] = S[100(c+1)] - S[100c]  (S[0] == 0)
                nc.vector.tensor_sub(
                    csall[:, cs_off : cs_off + nchunk],
                    sc[:, CHUNK : w + 1 : CHUNK],
                    sc[:, 0:w:CHUNK],
                )
                vector.drain()
                cs_off += nchunk
            # partials = sum_c cs[c]^2  (SCALE^2 applied on host)
            nc.vector.tensor_mul(sqall[:], csall[:], csall[:])
            vector.drain()
            nc.vector.tensor_reduce(
                out=partials[:],
                in_=sqall[:],
                axis=mybir.AxisListType.X,
                op=mybir.AluOpType.add,
            ).then_inc(s_done, 1)

    return nc


def _get_nc():
    global _CACHED_NC
    if _CACHED_NC is None:
        _CACHED_NC = _build_nc()
    return _CACHED_NC


def run_sharded(input, style, **run_kwargs):
    nc = _get_nc()
    xi = np.ascontiguousarray(np.asarray(input, dtype=np.float32)).reshape(
        N_ROWS, K
    )
    xs = np.ascontiguousarray(np.asarray(style, dtype=np.float32)).reshape(
        N_ROWS, K
    )
    in_maps = [
        {
            "input": xi[i * ROWS_PER_CORE : (i + 1) * ROWS_PER_CORE],
            "style": xs[i * ROWS_PER_CORE : (i + 1) * ROWS_PER_CORE],
        }
        for i in range(N_CORES)
    ]
    res = run_bass_kernel_spmd(nc, in_maps, list(range(N_CORES)), **run_kwargs)
    total = np.float64(0.0)
    for r in res.results:
        total += r["out"].astype(np.float64).sum()
    return np.array(total * SCALE2, dtype=np.float32), res


def kernel(input, style):
    loss, _ = run_sharded(input, style)
    return loss
